# revision 40
# baseline (speedup 1.0000x reference)
"""Trainium2 Bass kernel for batched multi-head attention.

Problem: query/key/value [B=2, H=16, S=2048, D=64] fp32, per-(b,h) divisor
`inv_scale_factor` [B, H, 1, 1].  out = softmax(Q K^T / inv_scale) V.

Sharding: the 32 (b,h) heads are split across 8 NeuronCores, 4 heads per
core, fully data-parallel (no collectives).  Each core runs the same
program on its own 4-head slice.

Per-core algorithm (per head, Sq tiled into q-blocks of 1024):
  - Load Q, K, V naturally ([128 seq, 64 d] tiles), cast to fp16 on DVE.
  - Transpose Q and K tiles on the PE as *regular* fp16 matmuls against an
    fp16 identity (out = tile.T @ I in fp32 PSUM, exact), giving Q^T / K^T
    with d on partitions; the PSUM->SBUF copy casts back to fp16 (exact).
  - scores_T[kv, q] = K^T_tile.T @ Q^T on the PE (fp16 in, fp32 PSUM).
  - P^T = exp(scores_T * (1/inv_scale) - ln 16) on the ACT engine straight
    out of PSUM with fp16 output.  The runtime per-head 1/inv_scale is a
    per-partition scale operand; the -ln 128 bias keeps exp and the
    unnormalized PV accumulator below fp16 max and cancels in the
    normalization.
    No max-subtraction pass is needed.
  - PV uses V augmented with a ones column ([kv, 65] fp16 stationary), so
    the softmax denominator (row 64) falls out of the same accumulating
    matmul chain that contracts P^T with V.
  - The [65, q] fp32 accumulator is copied to SBUF as fp16, transposed
    back on the PE (regular K=128 fp16 matmul against the identity), and
    each [128 q, 64 d] tile is scaled by 1/denominator (DVE reciprocal +
    per-partition tensor_scalar).
"""

import numpy as np

import concourse.bass as bass
import concourse.tile as tile
from concourse import bacc, mybir
from concourse.bass_utils import run_bass_kernel_spmd
from concourse.masks import make_identity

F32 = mybir.dt.float32
F16 = mybir.dt.float16
EXP = mybir.ActivationFunctionType.Exp
LNP = float(np.log(128.0))

B, H, SQ, SKV, D = 2, 16, 2048, 2048, 64
N_CORES = 8
HEADS_PER_CORE = (B * H) // N_CORES  # 4


def build_attention(nh=HEADS_PER_CORE, sq=SQ, skv=SKV, d=D, qblock=1024,
                    num_devices=N_CORES, enable_asserts=False):
    """Build the per-core Bass program. Returns the compiled Bacc module."""
    assert d == 64
    assert sq % 128 == 0 and skv % 128 == 0
    qblock = min(qblock, sq)
    assert sq % qblock == 0
    nchunk = min(512, qblock)          # matmul moving free-dim chunk
    assert qblock % nchunk == 0
    ntq = sq // 128                    # q tiles per head
    nkv = skv // 128                   # kv tiles per head
    nqb = sq // qblock                 # q blocks per head
    ntq_b = qblock // 128              # q tiles per q block

    nc = bacc.Bacc("TRN2", target_bir_lowering=False, debug=False,
                   enable_asserts=enable_asserts, num_devices=num_devices)

    q_dram = nc.dram_tensor("query", [nh, sq, d], F32, kind="ExternalInput").ap()
    k_dram = nc.dram_tensor("key", [nh, skv, d], F32, kind="ExternalInput").ap()
    v_dram = nc.dram_tensor("value", [nh, skv, d], F32, kind="ExternalInput").ap()
    inv_dram = nc.dram_tensor("inv_scale", [1, nh], F32, kind="ExternalInput").ap()
    o_dram = nc.dram_tensor("out", [nh, sq, d], F32, kind="ExternalOutput").ap()

    with tile.TileContext(nc) as tc:
        _attention_body(tc, o_dram, q_dram, k_dram, v_dram, inv_dram,
                        nh, sq, skv, d, qblock, nchunk, ntq, nkv, nqb, ntq_b)

    nc.compile()
    return nc


def _attention_body(tc, o_dram, q_dram, k_dram, v_dram, inv_dram,
                    nh, sq, skv, d, qblock, nchunk, ntq, nkv, nqb, ntq_b):
    nc = tc.nc
    from contextlib import ExitStack
    with ExitStack() as ctx:
        const = ctx.enter_context(tc.tile_pool(name="const", bufs=1))
        qnatp = ctx.enter_context(tc.tile_pool(name="qnat", bufs=2))
        knatp = ctx.enter_context(tc.tile_pool(name="knat", bufs=2))
        vnatp = ctx.enter_context(tc.tile_pool(name="vnat", bufs=2))
        qhp = ctx.enter_context(tc.tile_pool(name="qh", bufs=2))
        khp = ctx.enter_context(tc.tile_pool(name="kh", bufs=2))
        qtp = ctx.enter_context(tc.tile_pool(name="qt", bufs=2))
        ktp = ctx.enter_context(tc.tile_pool(name="kt", bufs=2))
        vaugp = ctx.enter_context(tc.tile_pool(name="vaug", bufs=2))
        ptp = ctx.enter_context(tc.tile_pool(name="pt", bufs=3))
        osbp = ctx.enter_context(tc.tile_pool(name="osb", bufs=2))
        finp = ctx.enter_context(tc.tile_pool(name="fin", bufs=2))
        recp = ctx.enter_context(tc.tile_pool(name="rec", bufs=4))
        scp = ctx.enter_context(tc.tile_pool(name="scps", bufs=2, space="PSUM"))
        outp = ctx.enter_context(tc.tile_pool(name="outps", bufs=1, space="PSUM"))
        tpp = ctx.enter_context(tc.tile_pool(name="tpps", bufs=2, space="PSUM"))

        # --- constants: identities, per-head 1/inv_scale broadcast [128, nh]
        ident = const.tile([128, 128], F32)
        make_identity(nc, ident[:])
        ident_h = const.tile([128, 128], F16)
        nc.vector.tensor_copy(ident_h[:], ident[:])
        inv_sb = const.tile([1, nh], F32)
        nc.sync.dma_start(inv_sb[:], inv_dram[:])
        recip_sb = const.tile([1, nh], F32)
        nc.vector.reciprocal(recip_sb[:], inv_sb[:])
        ones_row = const.tile([1, 128], F32)
        nc.vector.memset(ones_row[:], 1.0)
        bias_col = const.tile([128, 1], F32)
        nc.vector.memset(bias_col[:], -LNP)
        bps = tpp.tile([128, 128], F32, tag="tp")
        nc.tensor.matmul(bps[0:128, 0:nh], ones_row[0:1, 0:128],
                         recip_sb[0:1, 0:nh], start=True, stop=True)
        scale_all = const.tile([128, nh], F32)
        nc.vector.tensor_copy(scale_all[:], bps[0:128, 0:nh])

        def stage_head_loads(h):
            """DMA + fp16 casts for head h; returns tensors + transpose
            closures (one PE transpose + DVE copy each) to be drained
            interleaved with the previous head's main loop."""
            # DMAs and casts split in halves so the first transposes can
            # start as soon as the first half lands (matters for head 0,
            # whose staging is not hidden behind a previous head).
            hq = ntq // 2 * d
            qnat = qnatp.tile([128, ntq * d], F32, tag="qnat", name="qnat")
            qdr = q_dram[h].rearrange("(t p) e -> p t e", p=128)
            qnv = qnat[:].rearrange("p (t e) -> p t e", e=d)
            knat = knatp.tile([128, nkv * d], F32, tag="knat", name="knat")
            kdr = k_dram[h].rearrange("(t p) e -> p t e", p=128)
            knv = knat[:].rearrange("p (t e) -> p t e", e=d)
            vnat = vnatp.tile([128, nkv * (d + 1)], F32, tag="vnat", name="vnat")
            nc.gpsimd.memset(vnat[:], 1.0)
            # queue order: Q half 1, K half 1, V, Q half 2, K half 2 — the
            # first QK + PV need (q-block 0, kt 0, vaug) as early as possible
            nq4 = max(1, ntq // 4)
            nk4 = max(1, nkv // 4)
            nc.sync.dma_start(qnv[:, 0:nq4, :], qdr[:, 0:nq4, :])
            nc.sync.dma_start(knv[:, 0:nk4, :], kdr[:, 0:nk4, :])
            nc.sync.dma_start(qnv[:, nq4:ntq // 2, :], qdr[:, nq4:ntq // 2, :])
            nc.sync.dma_start(knv[:, nk4:nkv // 2, :], kdr[:, nk4:nkv // 2, :])
            nc.sync.dma_start(
                vnat[:].rearrange("p (t e) -> p t e", e=d + 1)[:, :, 0:d],
                v_dram[h].rearrange("(t p) e -> p t e", p=128))
            nc.sync.dma_start(qnv[:, ntq // 2:, :], qdr[:, ntq // 2:, :])
            nc.sync.dma_start(knv[:, nkv // 2:, :], kdr[:, nkv // 2:, :])
            # the fp16 cast of Q also applies 1/inv_scale, so the exp's scale
            # operand is an immediate (an AP scale costs ~110ns per ACTIVATE)
            sh = scale_all[:, h:h + 1]
            qh16 = qhp.tile([128, ntq * d], F16, tag="qh", name="qh16")
            nc.vector.tensor_scalar_mul(qh16[:, 0:nq4 * d], qnat[:, 0:nq4 * d], sh)
            nc.vector.tensor_scalar_mul(qh16[:, nq4 * d:hq], qnat[:, nq4 * d:hq], sh)
            nc.vector.tensor_scalar_mul(qh16[:, hq:], qnat[:, hq:], sh)
            hk = nkv // 2 * d
            kh16 = khp.tile([128, nkv * d], F16, tag="kh", name="kh16")
            nc.vector.tensor_copy(kh16[:, 0:nk4 * d], knat[:, 0:nk4 * d])
            nc.vector.tensor_copy(kh16[:, nk4 * d:hk], knat[:, nk4 * d:hk])
            nc.vector.tensor_copy(kh16[:, hk:], knat[:, hk:])
            vaug = vaugp.tile([128, nkv * (d + 1)], F16, tag="vaug", name="vaug")
            nc.vector.tensor_copy(vaug[:], vnat[:])

            # Q^T, K^T via regular fp16 matmuls against identity (exact).
            # Rows 64:128 are zero-filled so QK^T can run with a full K=128
            # contraction (zeros contribute nothing): K=64 matmuls keep only
            # half the PE rows active and the clock gate never un-throttles
            # (1.2 GHz); full-row matmuls warm the array to 2.4 GHz.
            # Rows 64:128 only ever hold zeros; pool slots rotate with period
            # 2, so after both slots are zeroed (heads 0 and 1) the reused
            # slots still hold zeros and the memset can be skipped.
            qt = qtp.tile([128, sq], F16, tag="qt", name="qt")
            kt = ktp.tile([128, skv], F16, tag="kt", name="kt")
            if h < 2:
                nc.vector.memset(qt[64:128, :], 0.0)
                nc.vector.memset(kt[64:128, :], 0.0)

            def tq(t):
                psq = tpp.tile([128, 128], F32, tag="tp", name="psq")
                nc.tensor.matmul(psq[0:64, 0:128],
                                 qh16[:, t * d:(t + 1) * d],
                                 ident_h[0:128, 0:128], start=True, stop=True)
                nc.vector.tensor_copy(qt[0:64, t * 128:(t + 1) * 128],
                                      psq[0:64, 0:128])

            def tk(t):
                psk = tpp.tile([128, 128], F32, tag="tp", name="psk")
                nc.tensor.matmul(psk[0:64, 0:128],
                                 kh16[:, t * d:(t + 1) * d],
                                 ident_h[0:128, 0:128], start=True, stop=True)
                nc.vector.tensor_copy(kt[0:64, t * 128:(t + 1) * 128],
                                      psk[0:64, 0:128])

            closures = [lambda t=t: tk(t) for t in range(nkv)]
            closures += [lambda t=t: tq(t) for t in range(ntq)]
            return qt, kt, vaug, closures

        # Head 0: drain only the transposes the first q-block needs (kt 0-2,
        # qt tiles of q-block 0); the rest interleave into its own main loop.
        staged = stage_head_loads(0)
        prefix = staged[3][0:nkv] + staged[3][nkv:nkv + ntq_b]
        rest = staged[3][nkv + ntq_b:]
        for f in prefix:
            f()
        staged = staged[:3] + (rest,)

        osb_count = [0]

        def make_epilogue(h, qb, out_ps):
            """Per-q-block epilogue as a list of small closures, drained one
            per kv-iteration so the PE/DVE work hides under ACT's exp.  The
            transpose back to [q, d] is a regular fp16 matmul against the
            identity with a full K=128 contraction (rows 65:128 of osb are
            zeroed once per pool slot) so it doesn't cool the PE clock."""
            cell = {}

            def c_copy():
                osb = osbp.tile([128, qblock], F16, tag="osb", name="osb")
                if osb_count[0] < 2:
                    nc.vector.memset(osb[64:128, :], 0.0)
                osb_count[0] += 1
                nc.vector.tensor_copy(osb[0:65, :], out_ps[0:65, :])
                fin = finp.tile([128, ntq_b * d], F32, tag="fin", name="fin")
                cell["osb"], cell["fin"] = osb, fin

            def c_tile(st):
                pso = tpp.tile([128, 128], F32, tag="tp", name="pso")
                nc.tensor.matmul(pso[0:128, 0:65],
                                 cell["osb"][0:128, st * 128:(st + 1) * 128],
                                 ident_h[0:128, 0:65], start=True, stop=True)
                rec = recp.tile([128, 1], F32, tag="rec", name="rec")
                nc.vector.reciprocal(rec[:], pso[:, 64:65])
                nc.vector.tensor_scalar_mul(
                    cell["fin"][:, st * d:(st + 1) * d], pso[:, 0:d], rec[:])

            def c_dma():
                nc.sync.dma_start(
                    o_dram[h].rearrange("(t p) e -> p t e", p=128)[
                        :, qb * ntq_b:(qb + 1) * ntq_b, :],
                    cell["fin"][:].rearrange("p (t e) -> p t e", e=d))

            return [c_copy] + [lambda st=st: c_tile(st) for st in range(ntq_b)] \
                + [c_dma]

        # ---------------- main loops ----------------
        # Per head, a flat (qb, kv) stream, software-pipelined in emission:
        #   QK(i+1), exp(i), PV(i)
        # so the in-order PE always has the next scores matmul queued while
        # ACT runs exp(i); ACT is the saturated engine.  Background `work`
        # (next head's staging transposes, previous q-block's epilogue) is
        # drained a bit per iteration into the PE/DVE slack so neither
        # q-block nor head boundaries bubble the ACT stream.
        stage_q = []   # next head's staging: MUST be empty before that head
        epi_q = []     # epilogue pieces: only self-dependent, may trail
        niter = nqb * nkv
        for h in range(nh):
            qt, kt, vaug, pending = staged
            stage_q.extend(pending)
            if h + 1 < nh:
                nxt = stage_head_loads(h + 1)
                stage_q.extend(nxt[3])
            else:
                nxt = None

            def emit_qk(it):
                qb, kvt = divmod(it, nkv)
                q0 = qb * qblock
                sc = scp.tile([128, qblock], F32, tag="sc", name="sc")
                for c in range(qblock // nchunk):
                    nc.tensor.matmul(
                        sc[:, c * nchunk:(c + 1) * nchunk],
                        kt[0:128, kvt * 128:(kvt + 1) * 128],
                        qt[0:128, q0 + c * nchunk:q0 + (c + 1) * nchunk],
                        start=True, stop=True)
                return sc

            sc_cur = emit_qk(0)
            out_ps = None
            for it in range(niter):
                qb, kvt = divmod(it, nkv)
                if kvt == 0:
                    out_ps = outp.tile([65, qblock], F32, tag="out",
                                       name="out_ps")
                sc_next = emit_qk(it + 1) if it + 1 < niter else None
                pt = ptp.tile([128, qblock], F16, tag="pt")
                nc.scalar.activation(pt[:], sc_cur[:], EXP,
                                     bias=bias_col[:], scale=1.0)
                for c in range(qblock // nchunk):
                    nc.tensor.matmul(
                        out_ps[0:65, c * nchunk:(c + 1) * nchunk],
                        vaug[:, kvt * (d + 1):(kvt + 1) * (d + 1)],
                        pt[:, c * nchunk:(c + 1) * nchunk],
                        start=(kvt == 0), stop=(kvt == nkv - 1))
                sc_cur = sc_next
                if kvt == nkv - 1:
                    eps = make_epilogue(h, qb, out_ps)
                    epi_q.insert(0, eps[0])  # the PSUM->SBUF copy frees the
                    epi_q.extend(eps[1:])    # accumulator slot: drain first
                budget = 2
                if stage_q:
                    stage_q.pop(0)()
                    budget -= 1
                while budget and stage_q and \
                        len(stage_q) > max(0, niter - 2 - it):
                    stage_q.pop(0)()
                    budget -= 1
                if budget and epi_q:
                    epi_q.pop(0)()
            while stage_q:
                stage_q.pop(0)()
            if nxt is not None:
                staged = nxt[:3] + ([],)

        while epi_q:
            epi_q.pop(0)()


_NC_CACHE = {}


def _get_program():
    key = "full"
    if key not in _NC_CACHE:
        _NC_CACHE[key] = build_attention()
    return _NC_CACHE[key]


def kernel(query, key, value, inv_scale_factor):
    """Full-input entry point: shard over 8 cores, run, gather."""
    nc = _get_program()
    q = np.ascontiguousarray(query, dtype=np.float32).reshape(B * H, SQ, D)
    k = np.ascontiguousarray(key, dtype=np.float32).reshape(B * H, SKV, D)
    v = np.ascontiguousarray(value, dtype=np.float32).reshape(B * H, SKV, D)
    inv = np.ascontiguousarray(inv_scale_factor, dtype=np.float32).reshape(B * H)

    hpc = HEADS_PER_CORE
    in_maps = []
    for c in range(N_CORES):
        s = slice(c * hpc, (c + 1) * hpc)
        in_maps.append({
            "query": q[s],
            "key": k[s],
            "value": v[s],
            "inv_scale": inv[s].reshape(1, hpc),
        })
    res = run_bass_kernel_spmd(nc, in_maps, core_ids=list(range(N_CORES)))
    out = np.concatenate([res.results[c]["out"] for c in range(N_CORES)], axis=0)
    return out.reshape(B, H, SQ, D)


# revision 41
# speedup vs baseline: 1.0399x; 1.0399x over previous
"""Trainium2 Bass kernel for batched multi-head attention.

Problem: query/key/value [B=2, H=16, S=2048, D=64] fp32, per-(b,h) divisor
`inv_scale_factor` [B, H, 1, 1].  out = softmax(Q K^T / inv_scale) V.

Sharding: the 32 (b,h) heads are split across 8 NeuronCores, 4 heads per
core, fully data-parallel (no collectives).  Each core runs the same
program on its own 4-head slice.

Per-core algorithm (per head, Sq tiled into q-blocks of 1024):
  - Load Q, K, V naturally ([128 seq, 64 d] tiles), cast to fp16 on DVE.
  - Transpose Q and K tiles on the PE as *regular* fp16 matmuls against an
    fp16 identity (out = tile.T @ I in fp32 PSUM, exact), giving Q^T / K^T
    with d on partitions; the PSUM->SBUF copy casts back to fp16 (exact).
  - scores_T[kv, q] = K^T_tile.T @ Q^T on the PE (fp16 in, fp32 PSUM).
  - P^T = exp(scores_T * (1/inv_scale) - ln 16) on the ACT engine straight
    out of PSUM with fp16 output.  The runtime per-head 1/inv_scale is a
    per-partition scale operand; the -ln 128 bias keeps exp and the
    unnormalized PV accumulator below fp16 max and cancels in the
    normalization.
    No max-subtraction pass is needed.
  - PV uses V augmented with a ones column ([kv, 65] fp16 stationary), so
    the softmax denominator (row 64) falls out of the same accumulating
    matmul chain that contracts P^T with V.
  - The [65, q] fp32 accumulator is copied to SBUF as fp16, transposed
    back on the PE (regular K=128 fp16 matmul against the identity), and
    each [128 q, 64 d] tile is scaled by 1/denominator (DVE reciprocal +
    per-partition tensor_scalar).
"""

import numpy as np

import concourse.bass as bass
import concourse.tile as tile
from concourse import bacc, mybir
from concourse.bass_utils import run_bass_kernel_spmd
from concourse.masks import make_identity

F32 = mybir.dt.float32
F16 = mybir.dt.float16
EXP = mybir.ActivationFunctionType.Exp
LNP = float(np.log(128.0))

B, H, SQ, SKV, D = 2, 16, 2048, 2048, 64
N_CORES = 8
HEADS_PER_CORE = (B * H) // N_CORES  # 4


def build_attention(nh=HEADS_PER_CORE, sq=SQ, skv=SKV, d=D, qblock=1024,
                    num_devices=N_CORES, enable_asserts=False):
    """Build the per-core Bass program. Returns the compiled Bacc module."""
    assert d == 64
    assert sq % 128 == 0 and skv % 128 == 0
    qblock = min(qblock, sq)
    assert sq % qblock == 0
    nchunk = min(512, qblock)          # matmul moving free-dim chunk
    assert qblock % nchunk == 0
    ntq = sq // 128                    # q tiles per head
    nkv = skv // 128                   # kv tiles per head
    nqb = sq // qblock                 # q blocks per head
    ntq_b = qblock // 128              # q tiles per q block

    nc = bacc.Bacc("TRN2", target_bir_lowering=False, debug=False,
                   enable_asserts=enable_asserts, num_devices=num_devices)

    q_dram = nc.dram_tensor("query", [nh, sq, d], F32, kind="ExternalInput").ap()
    k_dram = nc.dram_tensor("key", [nh, skv, d], F32, kind="ExternalInput").ap()
    v_dram = nc.dram_tensor("value", [nh, skv, d], F32, kind="ExternalInput").ap()
    inv_dram = nc.dram_tensor("inv_scale", [1, nh], F32, kind="ExternalInput").ap()
    o_dram = nc.dram_tensor("out", [nh, sq, d], F32, kind="ExternalOutput").ap()

    with tile.TileContext(nc) as tc:
        _attention_body(tc, o_dram, q_dram, k_dram, v_dram, inv_dram,
                        nh, sq, skv, d, qblock, nchunk, ntq, nkv, nqb, ntq_b)

    nc.compile()
    return nc


def _attention_body(tc, o_dram, q_dram, k_dram, v_dram, inv_dram,
                    nh, sq, skv, d, qblock, nchunk, ntq, nkv, nqb, ntq_b):
    nc = tc.nc
    from contextlib import ExitStack
    with ExitStack() as ctx:
        const = ctx.enter_context(tc.tile_pool(name="const", bufs=1))
        qnatp = ctx.enter_context(tc.tile_pool(name="qnat", bufs=2))
        knatp = ctx.enter_context(tc.tile_pool(name="knat", bufs=2))
        vnatp = ctx.enter_context(tc.tile_pool(name="vnat", bufs=2))
        qhp = ctx.enter_context(tc.tile_pool(name="qh", bufs=2))
        khp = ctx.enter_context(tc.tile_pool(name="kh", bufs=2))
        qtp = ctx.enter_context(tc.tile_pool(name="qt", bufs=2))
        ktp = ctx.enter_context(tc.tile_pool(name="kt", bufs=2))
        vaugp = ctx.enter_context(tc.tile_pool(name="vaug", bufs=2))
        ptp = ctx.enter_context(tc.tile_pool(name="pt", bufs=3))
        osbp = ctx.enter_context(tc.tile_pool(name="osb", bufs=2))
        finp = ctx.enter_context(tc.tile_pool(name="fin", bufs=2))
        recp = ctx.enter_context(tc.tile_pool(name="rec", bufs=4))
        scp = ctx.enter_context(tc.tile_pool(name="scps", bufs=2, space="PSUM"))
        outp = ctx.enter_context(tc.tile_pool(name="outps", bufs=1, space="PSUM"))
        tpp = ctx.enter_context(tc.tile_pool(name="tpps", bufs=2, space="PSUM"))

        # --- constants: identities, per-head 1/inv_scale broadcast [128, nh]
        ident = const.tile([128, 128], F32)
        make_identity(nc, ident[:])
        ident_h = const.tile([128, 128], F16)
        nc.vector.tensor_copy(ident_h[:], ident[:])
        inv_sb = const.tile([1, nh], F32)
        nc.sync.dma_start(inv_sb[:], inv_dram[:])
        recip_sb = const.tile([1, nh], F32)
        nc.vector.reciprocal(recip_sb[:], inv_sb[:])
        ones_row = const.tile([1, 128], F32)
        nc.vector.memset(ones_row[:], 1.0)
        bias_col = const.tile([128, 1], F32)
        nc.vector.memset(bias_col[:], -LNP)
        bps = tpp.tile([128, 128], F32, tag="tp")
        nc.tensor.matmul(bps[0:128, 0:nh], ones_row[0:1, 0:128],
                         recip_sb[0:1, 0:nh], start=True, stop=True)
        scale_all = const.tile([128, nh], F32)
        nc.vector.tensor_copy(scale_all[:], bps[0:128, 0:nh])

        def stage_head_loads(h):
            """DMA + fp16 casts for head h; returns tensors + transpose
            closures (one PE transpose + DVE copy each) to be drained
            interleaved with the previous head's main loop."""
            # DMAs and casts split in halves so the first transposes can
            # start as soon as the first half lands (matters for head 0,
            # whose staging is not hidden behind a previous head).
            hq = ntq // 2 * d
            qnat = qnatp.tile([128, ntq * d], F32, tag="qnat", name="qnat")
            qdr = q_dram[h].rearrange("(t p) e -> p t e", p=128)
            qnv = qnat[:].rearrange("p (t e) -> p t e", e=d)
            knat = knatp.tile([128, nkv * d], F32, tag="knat", name="knat")
            kdr = k_dram[h].rearrange("(t p) e -> p t e", p=128)
            knv = knat[:].rearrange("p (t e) -> p t e", e=d)
            vnat = vnatp.tile([128, nkv * (d + 1)], F32, tag="vnat", name="vnat")
            nc.gpsimd.memset(vnat[:], 1.0)
            # queue order: Q half 1, K half 1, V, Q half 2, K half 2 — the
            # first QK + PV need (q-block 0, kt 0, vaug) as early as possible
            nq4 = max(1, ntq // 4)
            nk4 = max(1, nkv // 4)
            nc.sync.dma_start(qnv[:, 0:nq4, :], qdr[:, 0:nq4, :])
            nc.sync.dma_start(knv[:, 0:nk4, :], kdr[:, 0:nk4, :])
            nc.sync.dma_start(qnv[:, nq4:ntq // 2, :], qdr[:, nq4:ntq // 2, :])
            nc.sync.dma_start(knv[:, nk4:nkv // 2, :], kdr[:, nk4:nkv // 2, :])
            nc.sync.dma_start(
                vnat[:].rearrange("p (t e) -> p t e", e=d + 1)[:, :, 0:d],
                v_dram[h].rearrange("(t p) e -> p t e", p=128))
            nc.sync.dma_start(qnv[:, ntq // 2:, :], qdr[:, ntq // 2:, :])
            nc.sync.dma_start(knv[:, nkv // 2:, :], kdr[:, nkv // 2:, :])
            # the fp16 cast of Q also applies 1/inv_scale, so the exp's scale
            # operand is an immediate (an AP scale costs ~110ns per ACTIVATE)
            sh = scale_all[:, h:h + 1]
            qh16 = qhp.tile([128, ntq * d], F16, tag="qh", name="qh16")
            nc.vector.tensor_scalar_mul(qh16[:, 0:nq4 * d], qnat[:, 0:nq4 * d], sh)
            nc.vector.tensor_scalar_mul(qh16[:, nq4 * d:hq], qnat[:, nq4 * d:hq], sh)
            nc.vector.tensor_scalar_mul(qh16[:, hq:], qnat[:, hq:], sh)
            hk = nkv // 2 * d
            kh16 = khp.tile([128, nkv * d], F16, tag="kh", name="kh16")
            nc.vector.tensor_copy(kh16[:, 0:nk4 * d], knat[:, 0:nk4 * d])
            nc.vector.tensor_copy(kh16[:, nk4 * d:hk], knat[:, nk4 * d:hk])
            nc.vector.tensor_copy(kh16[:, hk:], knat[:, hk:])
            vaug = vaugp.tile([128, nkv * (d + 1)], F16, tag="vaug", name="vaug")
            nc.vector.tensor_copy(vaug[:], vnat[:])

            # Q^T, K^T via regular fp16 matmuls against identity (exact).
            # Rows 64:128 are zero-filled so QK^T can run with a full K=128
            # contraction (zeros contribute nothing): K=64 matmuls keep only
            # half the PE rows active and the clock gate never un-throttles
            # (1.2 GHz); full-row matmuls warm the array to 2.4 GHz.
            # Rows 64:128 only ever hold zeros; pool slots rotate with period
            # 2, so after both slots are zeroed (heads 0 and 1) the reused
            # slots still hold zeros and the memset can be skipped.
            qt = qtp.tile([128, sq], F16, tag="qt", name="qt")
            kt = ktp.tile([128, skv], F16, tag="kt", name="kt")
            if h < 2:
                nc.vector.memset(qt[64:128, :], 0.0)
                nc.vector.memset(kt[64:128, :], 0.0)

            def tq(t):
                psq = tpp.tile([128, 128], F32, tag="tp", name="psq")
                nc.tensor.matmul(psq[0:64, 0:128],
                                 qh16[:, t * d:(t + 1) * d],
                                 ident_h[0:128, 0:128], start=True, stop=True)
                nc.vector.tensor_copy(qt[0:64, t * 128:(t + 1) * 128],
                                      psq[0:64, 0:128])

            def tk(t):
                psk = tpp.tile([128, 128], F32, tag="tp", name="psk")
                nc.tensor.matmul(psk[0:64, 0:128],
                                 kh16[:, t * d:(t + 1) * d],
                                 ident_h[0:128, 0:128], start=True, stop=True)
                nc.vector.tensor_copy(kt[0:64, t * 128:(t + 1) * 128],
                                      psk[0:64, 0:128])

            closures = [lambda t=t: tk(t) for t in range(nkv)]
            closures += [lambda t=t: tq(t) for t in range(ntq)]
            return qt, kt, vaug, closures

        # Head 0: drain only the transposes the first q-block needs (kt 0-2,
        # qt tiles of q-block 0); the rest interleave into its own main loop.
        staged = stage_head_loads(0)
        nk0 = min(6, nkv)
        prefix = staged[3][0:nk0] + staged[3][nkv:nkv + ntq_b]
        rest = staged[3][nk0:nkv] + staged[3][nkv + ntq_b:]
        for f in prefix:
            f()
        staged = staged[:3] + (rest,)

        osb_count = [0]

        def make_epilogue(h, qb, out_ps):
            """Per-q-block epilogue as a list of small closures, drained one
            per kv-iteration so the PE/DVE work hides under ACT's exp.  The
            transpose back to [q, d] is a regular fp16 matmul against the
            identity with a full K=128 contraction (rows 65:128 of osb are
            zeroed once per pool slot) so it doesn't cool the PE clock."""
            cell = {}

            def c_copy():
                osb = osbp.tile([128, qblock], F16, tag="osb", name="osb")
                if osb_count[0] < 2:
                    nc.vector.memset(osb[64:128, :], 0.0)
                osb_count[0] += 1
                nc.vector.tensor_copy(osb[0:65, :], out_ps[0:65, :])
                fin = finp.tile([128, ntq_b * d], F32, tag="fin", name="fin")
                cell["osb"], cell["fin"] = osb, fin

            def c_tile(st):
                pso = tpp.tile([128, 128], F32, tag="tp", name="pso")
                nc.tensor.matmul(pso[0:128, 0:65],
                                 cell["osb"][0:128, st * 128:(st + 1) * 128],
                                 ident_h[0:128, 0:65], start=True, stop=True)
                rec = recp.tile([128, 1], F32, tag="rec", name="rec")
                nc.vector.reciprocal(rec[:], pso[:, 64:65])
                nc.vector.tensor_scalar_mul(
                    cell["fin"][:, st * d:(st + 1) * d], pso[:, 0:d], rec[:])

            def c_dma():
                nc.sync.dma_start(
                    o_dram[h].rearrange("(t p) e -> p t e", p=128)[
                        :, qb * ntq_b:(qb + 1) * ntq_b, :],
                    cell["fin"][:].rearrange("p (t e) -> p t e", e=d))

            return [c_copy] + [lambda st=st: c_tile(st) for st in range(ntq_b)] \
                + [c_dma]

        # ---------------- main loops ----------------
        # Per head, a flat (qb, kv) stream, software-pipelined in emission:
        #   QK(i+1), exp(i), PV(i)
        # so the in-order PE always has the next scores matmul queued while
        # ACT runs exp(i); ACT is the saturated engine.  Background `work`
        # (next head's staging transposes, previous q-block's epilogue) is
        # drained a bit per iteration into the PE/DVE slack so neither
        # q-block nor head boundaries bubble the ACT stream.
        stage_q = []   # next head's staging: MUST be empty before that head
        epi_q = []     # epilogue pieces: only self-dependent, may trail
        niter = nqb * nkv
        for h in range(nh):
            qt, kt, vaug, pending = staged
            stage_q.extend(pending)
            if h + 1 < nh:
                nxt = stage_head_loads(h + 1)
                stage_q.extend(nxt[3])
            else:
                nxt = None

            def emit_qk(it):
                qb, kvt = divmod(it, nkv)
                q0 = qb * qblock
                sc = scp.tile([128, qblock], F32, tag="sc", name="sc")
                for c in range(qblock // nchunk):
                    nc.tensor.matmul(
                        sc[:, c * nchunk:(c + 1) * nchunk],
                        kt[0:128, kvt * 128:(kvt + 1) * 128],
                        qt[0:128, q0 + c * nchunk:q0 + (c + 1) * nchunk],
                        start=True, stop=True)
                return sc

            sc_cur = emit_qk(0)
            out_ps = None
            for it in range(niter):
                qb, kvt = divmod(it, nkv)
                if kvt == 0:
                    out_ps = outp.tile([65, qblock], F32, tag="out",
                                       name="out_ps")
                sc_next = emit_qk(it + 1) if it + 1 < niter else None
                pt = ptp.tile([128, qblock], F16, tag="pt")
                nc.scalar.activation(pt[:], sc_cur[:], EXP,
                                     bias=bias_col[:], scale=1.0)
                for c in range(qblock // nchunk):
                    nc.tensor.matmul(
                        out_ps[0:65, c * nchunk:(c + 1) * nchunk],
                        vaug[:, kvt * (d + 1):(kvt + 1) * (d + 1)],
                        pt[:, c * nchunk:(c + 1) * nchunk],
                        start=(kvt == 0), stop=(kvt == nkv - 1))
                sc_cur = sc_next
                if kvt == nkv - 1:
                    eps = make_epilogue(h, qb, out_ps)
                    epi_q.insert(0, eps[0])  # the PSUM->SBUF copy frees the
                    epi_q.extend(eps[1:])    # accumulator slot: drain first
                budget = 2
                while budget and stage_q and \
                        len(stage_q) > max(0, niter - 2 - it):
                    stage_q.pop(0)()
                    budget -= 1
                if budget and stage_q:
                    stage_q.pop(0)()
                    budget -= 1
                if budget and epi_q:
                    epi_q.pop(0)()
            while stage_q:
                stage_q.pop(0)()
            if nxt is not None:
                staged = nxt[:3] + ([],)

        while epi_q:
            epi_q.pop(0)()


_NC_CACHE = {}


def _get_program():
    key = "full"
    if key not in _NC_CACHE:
        _NC_CACHE[key] = build_attention()
    return _NC_CACHE[key]


def kernel(query, key, value, inv_scale_factor):
    """Full-input entry point: shard over 8 cores, run, gather."""
    nc = _get_program()
    q = np.ascontiguousarray(query, dtype=np.float32).reshape(B * H, SQ, D)
    k = np.ascontiguousarray(key, dtype=np.float32).reshape(B * H, SKV, D)
    v = np.ascontiguousarray(value, dtype=np.float32).reshape(B * H, SKV, D)
    inv = np.ascontiguousarray(inv_scale_factor, dtype=np.float32).reshape(B * H)

    hpc = HEADS_PER_CORE
    in_maps = []
    for c in range(N_CORES):
        s = slice(c * hpc, (c + 1) * hpc)
        in_maps.append({
            "query": q[s],
            "key": k[s],
            "value": v[s],
            "inv_scale": inv[s].reshape(1, hpc),
        })
    res = run_bass_kernel_spmd(nc, in_maps, core_ids=list(range(N_CORES)))
    out = np.concatenate([res.results[c]["out"] for c in range(N_CORES)], axis=0)
    return out.reshape(B, H, SQ, D)


# revision 42
# speedup vs baseline: 1.0478x; 1.0077x over previous
"""Trainium2 Bass kernel for batched multi-head attention.

Problem: query/key/value [B=2, H=16, S=2048, D=64] fp32, per-(b,h) divisor
`inv_scale_factor` [B, H, 1, 1].  out = softmax(Q K^T / inv_scale) V.

Sharding: the 32 (b,h) heads are split across 8 NeuronCores, 4 heads per
core, fully data-parallel (no collectives).  Each core runs the same
program on its own 4-head slice.

Per-core algorithm (per head, Sq tiled into q-blocks of 1024):
  - Load Q, K, V naturally ([128 seq, 64 d] tiles), cast to fp16 on DVE.
  - Transpose Q and K tiles on the PE as *regular* fp16 matmuls against an
    fp16 identity (out = tile.T @ I in fp32 PSUM, exact), giving Q^T / K^T
    with d on partitions; the PSUM->SBUF copy casts back to fp16 (exact).
  - scores_T[kv, q] = K^T_tile.T @ Q^T on the PE (fp16 in, fp32 PSUM).
  - P^T = exp(scores_T * (1/inv_scale) - ln 16) on the ACT engine straight
    out of PSUM with fp16 output.  The runtime per-head 1/inv_scale is a
    per-partition scale operand; the -ln 128 bias keeps exp and the
    unnormalized PV accumulator below fp16 max and cancels in the
    normalization.
    No max-subtraction pass is needed.
  - PV uses V augmented with a ones column ([kv, 65] fp16 stationary), so
    the softmax denominator (row 64) falls out of the same accumulating
    matmul chain that contracts P^T with V.
  - The [65, q] fp32 accumulator is copied to SBUF as fp16, transposed
    back on the PE (regular K=128 fp16 matmul against the identity), and
    each [128 q, 64 d] tile is scaled by 1/denominator (DVE reciprocal +
    per-partition tensor_scalar).
"""

import numpy as np

import concourse.bass as bass
import concourse.tile as tile
from concourse import bacc, mybir
from concourse.bass_utils import run_bass_kernel_spmd
from concourse.masks import make_identity

F32 = mybir.dt.float32
F16 = mybir.dt.float16
EXP = mybir.ActivationFunctionType.Exp
LNP = float(np.log(128.0))

B, H, SQ, SKV, D = 2, 16, 2048, 2048, 64
N_CORES = 8
HEADS_PER_CORE = (B * H) // N_CORES  # 4


def build_attention(nh=HEADS_PER_CORE, sq=SQ, skv=SKV, d=D, qblock=1024,
                    num_devices=N_CORES, enable_asserts=False):
    """Build the per-core Bass program. Returns the compiled Bacc module."""
    assert d == 64
    assert sq % 128 == 0 and skv % 128 == 0
    qblock = min(qblock, sq)
    assert sq % qblock == 0
    nchunk = min(512, qblock)          # matmul moving free-dim chunk
    assert qblock % nchunk == 0
    ntq = sq // 128                    # q tiles per head
    nkv = skv // 128                   # kv tiles per head
    nqb = sq // qblock                 # q blocks per head
    ntq_b = qblock // 128              # q tiles per q block

    nc = bacc.Bacc("TRN2", target_bir_lowering=False, debug=False,
                   enable_asserts=enable_asserts, num_devices=num_devices)

    q_dram = nc.dram_tensor("query", [nh, sq, d], F32, kind="ExternalInput").ap()
    k_dram = nc.dram_tensor("key", [nh, skv, d], F32, kind="ExternalInput").ap()
    v_dram = nc.dram_tensor("value", [nh, skv, d], F32, kind="ExternalInput").ap()
    inv_dram = nc.dram_tensor("inv_scale", [1, nh], F32, kind="ExternalInput").ap()
    o_dram = nc.dram_tensor("out", [nh, sq, d], F32, kind="ExternalOutput").ap()

    with tile.TileContext(nc) as tc:
        _attention_body(tc, o_dram, q_dram, k_dram, v_dram, inv_dram,
                        nh, sq, skv, d, qblock, nchunk, ntq, nkv, nqb, ntq_b)

    nc.compile()
    return nc


def _attention_body(tc, o_dram, q_dram, k_dram, v_dram, inv_dram,
                    nh, sq, skv, d, qblock, nchunk, ntq, nkv, nqb, ntq_b):
    nc = tc.nc
    from contextlib import ExitStack
    with ExitStack() as ctx:
        const = ctx.enter_context(tc.tile_pool(name="const", bufs=1))
        qnatp = ctx.enter_context(tc.tile_pool(name="qnat", bufs=2))
        knatp = ctx.enter_context(tc.tile_pool(name="knat", bufs=2))
        vnatp = ctx.enter_context(tc.tile_pool(name="vnat", bufs=2))
        qhp = ctx.enter_context(tc.tile_pool(name="qh", bufs=2))
        khp = ctx.enter_context(tc.tile_pool(name="kh", bufs=2))
        qtp = ctx.enter_context(tc.tile_pool(name="qt", bufs=2))
        ktp = ctx.enter_context(tc.tile_pool(name="kt", bufs=2))
        vaugp = ctx.enter_context(tc.tile_pool(name="vaug", bufs=2))
        ptp = ctx.enter_context(tc.tile_pool(name="pt", bufs=4))
        osbp = ctx.enter_context(tc.tile_pool(name="osb", bufs=2))
        finp = ctx.enter_context(tc.tile_pool(name="fin", bufs=2))
        recp = ctx.enter_context(tc.tile_pool(name="rec", bufs=4))
        scp = ctx.enter_context(tc.tile_pool(name="scps", bufs=2, space="PSUM"))
        outp = ctx.enter_context(tc.tile_pool(name="outps", bufs=1, space="PSUM"))
        tpp = ctx.enter_context(tc.tile_pool(name="tpps", bufs=2, space="PSUM"))

        # --- constants: identities, per-head 1/inv_scale broadcast [128, nh]
        ident = const.tile([128, 128], F32)
        make_identity(nc, ident[:])
        ident_h = const.tile([128, 128], F16)
        nc.vector.tensor_copy(ident_h[:], ident[:])
        inv_sb = const.tile([1, nh], F32)
        nc.sync.dma_start(inv_sb[:], inv_dram[:])
        recip_sb = const.tile([1, nh], F32)
        nc.vector.reciprocal(recip_sb[:], inv_sb[:])
        ones_row = const.tile([1, 128], F32)
        nc.vector.memset(ones_row[:], 1.0)
        bias_col = const.tile([128, 1], F32)
        nc.vector.memset(bias_col[:], -LNP)
        bps = tpp.tile([128, 128], F32, tag="tp")
        nc.tensor.matmul(bps[0:128, 0:nh], ones_row[0:1, 0:128],
                         recip_sb[0:1, 0:nh], start=True, stop=True)
        scale_all = const.tile([128, nh], F32)
        nc.vector.tensor_copy(scale_all[:], bps[0:128, 0:nh])

        def stage_head_loads(h):
            """DMA + fp16 casts for head h; returns tensors + transpose
            closures (one PE transpose + DVE copy each) to be drained
            interleaved with the previous head's main loop."""
            # DMAs and casts split in halves so the first transposes can
            # start as soon as the first half lands (matters for head 0,
            # whose staging is not hidden behind a previous head).
            hq = ntq // 2 * d
            qnat = qnatp.tile([128, ntq * d], F32, tag="qnat", name="qnat")
            qdr = q_dram[h].rearrange("(t p) e -> p t e", p=128)
            qnv = qnat[:].rearrange("p (t e) -> p t e", e=d)
            knat = knatp.tile([128, nkv * d], F32, tag="knat", name="knat")
            kdr = k_dram[h].rearrange("(t p) e -> p t e", p=128)
            knv = knat[:].rearrange("p (t e) -> p t e", e=d)
            vnat = vnatp.tile([128, nkv * (d + 1)], F32, tag="vnat", name="vnat")
            nc.gpsimd.memset(vnat[:], 1.0)
            # queue order: Q half 1, K half 1, V, Q half 2, K half 2 — the
            # first QK + PV need (q-block 0, kt 0, vaug) as early as possible
            nq4 = max(1, ntq // 4)
            nk4 = max(1, nkv // 4)
            nc.sync.dma_start(qnv[:, 0:nq4, :], qdr[:, 0:nq4, :])
            nc.sync.dma_start(knv[:, 0:nk4, :], kdr[:, 0:nk4, :])
            nc.sync.dma_start(qnv[:, nq4:ntq // 2, :], qdr[:, nq4:ntq // 2, :])
            nc.sync.dma_start(knv[:, nk4:nkv // 2, :], kdr[:, nk4:nkv // 2, :])
            nc.sync.dma_start(
                vnat[:].rearrange("p (t e) -> p t e", e=d + 1)[:, :, 0:d],
                v_dram[h].rearrange("(t p) e -> p t e", p=128))
            nc.sync.dma_start(qnv[:, ntq // 2:, :], qdr[:, ntq // 2:, :])
            nc.sync.dma_start(knv[:, nkv // 2:, :], kdr[:, nkv // 2:, :])
            # the fp16 cast of Q also applies 1/inv_scale, so the exp's scale
            # operand is an immediate (an AP scale costs ~110ns per ACTIVATE)
            sh = scale_all[:, h:h + 1]
            qh16 = qhp.tile([128, ntq * d], F16, tag="qh", name="qh16")
            nc.vector.tensor_scalar_mul(qh16[:, 0:nq4 * d], qnat[:, 0:nq4 * d], sh)
            nc.vector.tensor_scalar_mul(qh16[:, nq4 * d:hq], qnat[:, nq4 * d:hq], sh)
            nc.vector.tensor_scalar_mul(qh16[:, hq:], qnat[:, hq:], sh)
            hk = nkv // 2 * d
            kh16 = khp.tile([128, nkv * d], F16, tag="kh", name="kh16")
            nc.vector.tensor_copy(kh16[:, 0:nk4 * d], knat[:, 0:nk4 * d])
            nc.vector.tensor_copy(kh16[:, nk4 * d:hk], knat[:, nk4 * d:hk])
            nc.vector.tensor_copy(kh16[:, hk:], knat[:, hk:])
            vaug = vaugp.tile([128, nkv * (d + 1)], F16, tag="vaug", name="vaug")
            nc.vector.tensor_copy(vaug[:], vnat[:])

            # Q^T, K^T via regular fp16 matmuls against identity (exact).
            # Rows 64:128 are zero-filled so QK^T can run with a full K=128
            # contraction (zeros contribute nothing): K=64 matmuls keep only
            # half the PE rows active and the clock gate never un-throttles
            # (1.2 GHz); full-row matmuls warm the array to 2.4 GHz.
            # Rows 64:128 only ever hold zeros; pool slots rotate with period
            # 2, so after both slots are zeroed (heads 0 and 1) the reused
            # slots still hold zeros and the memset can be skipped.
            qt = qtp.tile([128, sq], F16, tag="qt", name="qt")
            kt = ktp.tile([128, skv], F16, tag="kt", name="kt")
            if h < 2:
                nc.vector.memset(qt[64:128, :], 0.0)
                nc.vector.memset(kt[64:128, :], 0.0)

            def tq(t):
                psq = tpp.tile([128, 128], F32, tag="tp", name="psq")
                nc.tensor.matmul(psq[0:64, 0:128],
                                 qh16[:, t * d:(t + 1) * d],
                                 ident_h[0:128, 0:128], start=True, stop=True)
                nc.vector.tensor_copy(qt[0:64, t * 128:(t + 1) * 128],
                                      psq[0:64, 0:128])

            def tk(t):
                psk = tpp.tile([128, 128], F32, tag="tp", name="psk")
                nc.tensor.matmul(psk[0:64, 0:128],
                                 kh16[:, t * d:(t + 1) * d],
                                 ident_h[0:128, 0:128], start=True, stop=True)
                nc.vector.tensor_copy(kt[0:64, t * 128:(t + 1) * 128],
                                      psk[0:64, 0:128])

            closures = [lambda t=t: tk(t) for t in range(nkv)]
            closures += [lambda t=t: tq(t) for t in range(ntq)]
            return qt, kt, vaug, closures

        # Head 0: drain only the transposes the first q-block needs (kt 0-2,
        # qt tiles of q-block 0); the rest interleave into its own main loop.
        staged = stage_head_loads(0)
        nk0 = min(6, nkv)
        prefix = staged[3][0:nk0] + staged[3][nkv:nkv + ntq_b]
        rest = staged[3][nk0:nkv] + staged[3][nkv + ntq_b:]
        for f in prefix:
            f()
        staged = staged[:3] + (rest,)

        osb_count = [0]

        def make_epilogue(h, qb, out_ps):
            """Per-q-block epilogue as a list of small closures, drained one
            per kv-iteration so the PE/DVE work hides under ACT's exp.  The
            transpose back to [q, d] is a regular fp16 matmul against the
            identity with a full K=128 contraction (rows 65:128 of osb are
            zeroed once per pool slot) so it doesn't cool the PE clock."""
            cell = {}

            def c_copy():
                osb = osbp.tile([128, qblock], F16, tag="osb", name="osb")
                if osb_count[0] < 2:
                    nc.vector.memset(osb[64:128, :], 0.0)
                osb_count[0] += 1
                nc.vector.tensor_copy(osb[0:65, :], out_ps[0:65, :])
                fin = finp.tile([128, ntq_b * d], F32, tag="fin", name="fin")
                cell["osb"], cell["fin"] = osb, fin

            def c_tile(st):
                pso = tpp.tile([128, 128], F32, tag="tp", name="pso")
                nc.tensor.matmul(pso[0:128, 0:65],
                                 cell["osb"][0:128, st * 128:(st + 1) * 128],
                                 ident_h[0:128, 0:65], start=True, stop=True)
                rec = recp.tile([128, 1], F32, tag="rec", name="rec")
                nc.vector.reciprocal(rec[:], pso[:, 64:65])
                nc.vector.tensor_scalar_mul(
                    cell["fin"][:, st * d:(st + 1) * d], pso[:, 0:d], rec[:])

            def c_dma():
                nc.sync.dma_start(
                    o_dram[h].rearrange("(t p) e -> p t e", p=128)[
                        :, qb * ntq_b:(qb + 1) * ntq_b, :],
                    cell["fin"][:].rearrange("p (t e) -> p t e", e=d))

            return [c_copy] + [lambda st=st: c_tile(st) for st in range(ntq_b)] \
                + [c_dma]

        # ---------------- main loops ----------------
        # Per head, a flat (qb, kv) stream, software-pipelined in emission:
        #   QK(i+1), exp(i), PV(i)
        # so the in-order PE always has the next scores matmul queued while
        # ACT runs exp(i); ACT is the saturated engine.  Background `work`
        # (next head's staging transposes, previous q-block's epilogue) is
        # drained a bit per iteration into the PE/DVE slack so neither
        # q-block nor head boundaries bubble the ACT stream.
        stage_q = []   # next head's staging: MUST be empty before that head
        epi_q = []     # epilogue pieces: only self-dependent, may trail
        niter = nqb * nkv
        for h in range(nh):
            qt, kt, vaug, pending = staged
            stage_q.extend(pending)
            if h + 1 < nh:
                nxt = stage_head_loads(h + 1)
                stage_q.extend(nxt[3])
            else:
                nxt = None

            def emit_qk(it):
                qb, kvt = divmod(it, nkv)
                q0 = qb * qblock
                sc = scp.tile([128, qblock], F32, tag="sc", name="sc")
                for c in range(qblock // nchunk):
                    nc.tensor.matmul(
                        sc[:, c * nchunk:(c + 1) * nchunk],
                        kt[0:128, kvt * 128:(kvt + 1) * 128],
                        qt[0:128, q0 + c * nchunk:q0 + (c + 1) * nchunk],
                        start=True, stop=True)
                return sc

            sc_cur = emit_qk(0)
            out_ps = None
            for it in range(niter):
                qb, kvt = divmod(it, nkv)
                if kvt == 0:
                    out_ps = outp.tile([65, qblock], F32, tag="out",
                                       name="out_ps")
                sc_next = emit_qk(it + 1) if it + 1 < niter else None
                pt = ptp.tile([128, qblock], F16, tag="pt")
                nc.scalar.activation(pt[:], sc_cur[:], EXP,
                                     bias=bias_col[:], scale=1.0)
                for c in range(qblock // nchunk):
                    nc.tensor.matmul(
                        out_ps[0:65, c * nchunk:(c + 1) * nchunk],
                        vaug[:, kvt * (d + 1):(kvt + 1) * (d + 1)],
                        pt[:, c * nchunk:(c + 1) * nchunk],
                        start=(kvt == 0), stop=(kvt == nkv - 1))
                sc_cur = sc_next
                if kvt == nkv - 1:
                    eps = make_epilogue(h, qb, out_ps)
                    epi_q.insert(0, eps[0])  # the PSUM->SBUF copy frees the
                    epi_q.extend(eps[1:])    # accumulator slot: drain first
                budget = 2
                while budget and stage_q and \
                        len(stage_q) > max(0, niter - 2 - it):
                    stage_q.pop(0)()
                    budget -= 1
                if budget and stage_q:
                    stage_q.pop(0)()
                    budget -= 1
                if budget and epi_q:
                    epi_q.pop(0)()
            while stage_q:
                stage_q.pop(0)()
            if nxt is not None:
                staged = nxt[:3] + ([],)

        while epi_q:
            epi_q.pop(0)()


_NC_CACHE = {}


def _get_program():
    key = "full"
    if key not in _NC_CACHE:
        _NC_CACHE[key] = build_attention()
    return _NC_CACHE[key]


def kernel(query, key, value, inv_scale_factor):
    """Full-input entry point: shard over 8 cores, run, gather."""
    nc = _get_program()
    q = np.ascontiguousarray(query, dtype=np.float32).reshape(B * H, SQ, D)
    k = np.ascontiguousarray(key, dtype=np.float32).reshape(B * H, SKV, D)
    v = np.ascontiguousarray(value, dtype=np.float32).reshape(B * H, SKV, D)
    inv = np.ascontiguousarray(inv_scale_factor, dtype=np.float32).reshape(B * H)

    hpc = HEADS_PER_CORE
    in_maps = []
    for c in range(N_CORES):
        s = slice(c * hpc, (c + 1) * hpc)
        in_maps.append({
            "query": q[s],
            "key": k[s],
            "value": v[s],
            "inv_scale": inv[s].reshape(1, hpc),
        })
    res = run_bass_kernel_spmd(nc, in_maps, core_ids=list(range(N_CORES)))
    out = np.concatenate([res.results[c]["out"] for c in range(N_CORES)], axis=0)
    return out.reshape(B, H, SQ, D)
